# revision 39
# baseline (speedup 1.0000x reference)
"""MoE grouped-linear kernel for Trainium2 (8 NeuronCores, expert-parallel).

y[t] = weight[expert_ids[t]] @ x[t] + bias[expert_ids[t]]
T=131072 tokens, E=64 experts, I=O=512, global per-expert capacity 3072
(overflow -> 0, matching the reference's capacity-bucketed dispatch).

Sharding: expert-parallel, count-adaptive. The host computes the routing
(argsort by expert), sorts experts by token count and assigns rank r to
(slot r//8, core r%8) so the 8 experts sharing a slot have similar counts;
slot k is compiled with nt[k] = ceil(max_count/128) token-tiles (the
program is built per nt-tuple and cached). Each expert's tokens are
gathered and pre-transposed on the host into the SBUF matmul layout
[128 i_lo, tile, 4 i_chunk, 128 tok_lo] fp16, so the device runs pure
dense GEMMs with no on-chip gather/scatter/transpose:

  per slot k (nt[k] token-tiles of 128):
    - one contiguous HWDGE load of X^T (SP ring; prefetched SKEW ahead;
      the first slot's load is split so matmuls start after ~0.75 MB),
    - per tile: 4 fp16 matmuls (X^T chunk stationary, W^T streaming,
      N=512) accumulate into one fp32 PSUM bank -- back-to-back warm
      matmuls at the 216 ns streaming roofline,
    - DVE evicts PSUM -> fp16 SBUF, fusing the fp32 bias add,
    - the result block is stored in ~6-tile chunks (ACT ring, separate
      from the SP load ring) so the final store tail is short.
  Bias is sent raw (16 KB) and replicated across partitions on-chip by
  K=1 matmuls that double as the PE warmup; slot-0 weights load at the
  head of the SP ring (the ACT ring gets ~1/4 bandwidth while big x
  descriptors stream), the rest just-in-time on the ACT ring.

The host scatters the fp16 result blocks back to token order and upcasts
to fp32. Tokens past a slot's device capacity (pos in [2304, 3072)) are
computed exactly on the host (~never happens for uniform routing); tokens
past the global capacity 3072 are 0 like the reference.
"""

import os
import sys

sys.path.insert(0, "/opt/trn_rl_repo")

import numpy as np

T, D, E, NC = 131072, 512, 64, 8
EL = E // NC      # experts per core (= number of slots)
CAPD = 2304       # max device per-expert capacity (18 tiles of 128)
NTMAX = CAPD // 128
CAPG = 3072       # reference global per-expert capacity
SKEW = 3          # x prefetch depth (slots)
P = 128

_cache = {}
last_result = None


def _build_program(nt_slot):
    from concourse import bacc, mybir, tile

    f32 = mybir.dt.float32
    f16 = mybir.dt.float16
    ntot = sum(nt_slot)
    off = [0]
    for nt in nt_slot:
        off.append(off[-1] + nt)

    nc = bacc.Bacc(
        "TRN2",
        target_bir_lowering=False,
        debug=False,
        enable_asserts=False,
        num_devices=NC,
    )
    x_d = nc.dram_tensor("x", [P, ntot * 512], f16, kind="ExternalInput").ap()
    w_d = nc.dram_tensor("wt", [P, EL * 4 * D], f16, kind="ExternalInput").ap()
    b_d = nc.dram_tensor("bias", [1, EL * D], f16, kind="ExternalInput").ap()
    y_d = nc.dram_tensor("y", [P, ntot * 512], f16, kind="ExternalOutput").ap()

    with tile.TileContext(nc) as tc:
        with (
            tc.tile_pool(name="const", bufs=1) as constp,
            tc.tile_pool(name="wt", bufs=3) as wtp,
            tc.tile_pool(name="bt", bufs=3) as btp,
            tc.tile_pool(name="xg0", bufs=1) as xg0p,
            tc.tile_pool(name="xg", bufs=SKEW + 1) as xgp,
            tc.tile_pool(name="ys", bufs=8) as ysp,
            tc.tile_pool(name="psY", bufs=6, space="PSUM") as psYp,
            tc.tile_pool(name="psB", bufs=2, space="PSUM") as psBp,
        ):
            def load_x(k):
                nt = nt_slot[k]
                if k == 0:
                    # split (2+6+rest tiles) so the first matmuls wait on
                    # only 0.25 MB and the rest streams in just-in-time
                    segs = []
                    b0 = 0
                    for si, n in enumerate([2, 6, nt]):
                        n = min(n, nt - b0)
                        if n <= 0:
                            break
                        t = xg0p.tile([P, n * 512], f16, tag=f"xg0{si}")
                        nc.sync.dma_start(
                            out=t[:], in_=x_d[:, b0 * 512 : (b0 + n) * 512]
                        )
                        segs.append((t, b0, n))
                        b0 += n
                    return segs
                t = xgp.tile([P, NTMAX * 512], f16, tag="xg")
                nc.sync.dma_start(
                    out=t[:, : nt * 512],
                    in_=x_d[:, off[k] * 512 : (off[k] + nt) * 512],
                )
                return [(t, 0, nt)]

            # prologue. SP ring: raw bias (16 KB), slot-0 weights (the ACT
            # ring gets ~1/4 of the bandwidth while big x loads stream),
            # the split slot-0 x load, then x prefetch. ACT ring: the
            # remaining weights just-in-time, then the y stores. All bias
            # replicates (K=1 matmuls off the raw-bias tile) run up front,
            # filling the dead window while x/w stream in.
            # HAM warmup operands: the clock gate only counts full-array
            # activity (K=1 matmuls don't flip it), so a run of zero-valued
            # K=128 matmuls off memset tiles warms the PE during the
            # initial DMA wait. They accumulate +0 into the slot-0 bias
            # replicate's PSUM group, which is read later — DCE-proof and
            # bit-exact.
            ones_t = constp.tile([1, P], f16)
            nc.gpsimd.memset(ones_t[:], 1.0)
            zeros_t = constp.tile([P, P], f16)
            nc.gpsimd.memset(zeros_t[:], 0.0)
            zjunk = constp.tile([P, D], f16)
            nc.gpsimd.memset(zjunk[:], 0.0)
            braw = constp.tile([1, EL * D], f16)
            nc.scalar.dma_start(out=braw[:], in_=b_d)

            WSKEW = 3

            def load_w(k, eng=None):
                w = wtp.tile([P, 4 * D], f16, tag="wt")
                (eng or nc.scalar).dma_start(
                    out=w[:], in_=w_d[:, k * 4 * D : (k + 1) * 4 * D]
                )
                return w

            # slot-0 SP-ring order: first 2 x tiles, then w0, then the rest
            # of x0 — the first tile matmuls and w0 both land early and the
            # mid chunk arrives just-in-time behind them
            nt0 = nt_slot[0]
            segs0 = []
            bb = 0
            for si, n in enumerate([2, 6, nt0]):
                n = min(n, nt0 - bb)
                if n <= 0:
                    break
                t0_ = xg0p.tile([P, n * 512], f16, tag=f"xg0{si}")
                nc.sync.dma_start(
                    out=t0_[:], in_=x_d[:, bb * 512 : (bb + n) * 512]
                )
                segs0.append((t0_, bb, n))
                bb += n
                if si == 0:
                    wpend = [load_w(0, eng=nc.sync)]
            pend = [segs0]
            for k in range(1, SKEW):
                pend.append(load_x(k))
            for k in range(1, WSKEW):
                wpend.append(load_w(k))

            def replicate_bias(k, warmup=0):
                psB = psBp.tile([P, D], f32, tag="psB")
                for r in range(warmup):
                    nc.tensor.matmul(
                        out=psB[:],
                        lhsT=zeros_t[:],
                        rhs=zjunk[:],
                        start=(r == 0),
                        stop=False,
                        skip_group_check=True,
                    )
                nc.tensor.matmul(
                    out=psB[:],
                    lhsT=ones_t[:],
                    rhs=braw[:, k * D : (k + 1) * D],
                    start=(warmup == 0),
                    stop=True,
                    skip_group_check=True,
                )
                b_k = btp.tile([P, D], f32, tag="bt")
                nc.vector.tensor_copy(out=b_k[:], in_=psB[:])
                return b_k

            b_next = replicate_bias(0, warmup=14)

            for k in range(EL):
                segs = pend.pop(0)
                w_k = wpend.pop(0)
                b_k = b_next
                nt = nt_slot[k]
                if k + 1 < EL:
                    b_next = replicate_bias(k + 1)
                # store-chunk boundaries; short final chunks on the last
                # slot so the kernel-tail store is small
                if k == EL - 1 and nt >= 14:
                    bnds = [6, 12, nt - 2, nt - 1, nt]
                else:
                    bnds = list(range(6, nt, 6)) + [nt]
                bset = set(bnds)
                ys = None
                done = 0
                for xt_t, bt0, nbt in segs:
                    for bi in range(nbt):
                        bt = bt0 + bi
                        if ys is None:
                            nxt = min(b for b in bnds if b > bt)
                            ys = ysp.tile([P, (nxt - bt) * D], f16, tag="ys")
                        psY = psYp.tile([P, D], f32, tag="psY")
                        for j in range(4):
                            nc.tensor.matmul(
                                out=psY[:],
                                lhsT=xt_t[:, bi * 512 + j * P : bi * 512 + (j + 1) * P],
                                rhs=w_k[:, j * D : (j + 1) * D],
                                start=(j == 0),
                                stop=(j == 3),
                            )
                        nc.vector.tensor_add(
                            out=ys[:, (bt - done) * D : (bt - done + 1) * D],
                            in0=psY[:],
                            in1=b_k[:],
                        )
                        if bt + 1 in bset:
                            nc.scalar.dma_start(
                                out=y_d[:, (off[k] + done) * 512 : (off[k] + bt + 1) * 512],
                                in_=ys[:, : (bt + 1 - done) * D],
                            )
                            done = bt + 1
                            ys = None
                if k + SKEW < EL:
                    pend.append(load_x(k + SKEW))
                if k + WSKEW < EL:
                    wpend.append(load_w(k + WSKEW))
    nc.compile()
    return nc


def _ensure_ntff_hook():
    """The agent image's antenv lacks axon_hooks; shim it and install the
    ctypes NTFF profiling hook so trace=True works under axon."""
    import types

    try:
        from antenv import axon_hooks  # noqa: F401
        return
    except ImportError:
        pass
    mod = types.ModuleType("antenv.axon_hooks")
    _h = {"hook": None}
    mod.set_axon_ntff_profile_hook = lambda h: _h.update(hook=h)
    mod.get_axon_ntff_profile_hook = lambda: _h["hook"]
    sys.modules["antenv.axon_hooks"] = mod
    import antenv

    antenv.axon_hooks = mod
    try:
        if "/root/.axon_site" not in sys.path:
            sys.path.insert(0, "/root/.axon_site")
        from trn_agent_boot.trn_boot import _ntff_profile_via_ctypes

        hook = _ntff_profile_via_ctypes("/opt/axon/libaxon_pjrt.so")
        if hook is not None:
            mod.set_axon_ntff_profile_hook(hook)
    except Exception:
        pass


def kernel(x, weight, bias, expert_ids):
    global last_result
    from concourse import bass_utils
    from concourse.bass_utils import run_bass_kernel_spmd

    x = np.asarray(x, dtype=np.float32)
    weight = np.asarray(weight, dtype=np.float32)
    bias = np.asarray(bias, dtype=np.float32)
    expert_ids = np.asarray(expert_ids, dtype=np.int32)

    # ---- host routing: tokens sorted by expert, position within expert ----
    order = np.argsort(expert_ids, kind="stable")
    ids_s = expert_ids[order]
    counts = np.bincount(expert_ids, minlength=E)
    starts = np.cumsum(counts) - counts
    pos_s = np.arange(T, dtype=np.int64) - starts[ids_s]
    sel = pos_s < CAPD  # tokens the device computes

    # sort experts by count desc; rank r -> (slot r//NC, core r%NC)
    counts_c = np.minimum(counts, CAPD)
    rank = np.argsort(-counts_c, kind="stable")
    perm = rank.reshape(EL, NC)  # perm[slot, core] = expert id
    nt_slot = tuple(
        max(1, int(-(-counts_c[perm[k]].max() // 128))) for k in range(EL)
    )
    off = [0]
    for nt in nt_slot:
        off.append(off[-1] + nt)
    ntot = off[-1]

    if nt_slot not in _cache:
        _cache[nt_slot] = _build_program(nt_slot)
    nc = _cache[nt_slot]

    # ---- pack x: [E, CAPD, D] fp16, then to [E, 128 i_lo, bt, j, 128 t_lo] ----
    x16 = x.astype(np.float16)
    buf = np.zeros((E, CAPD, D), np.float16)
    buf[ids_s[sel], pos_s[sel]] = x16[order[sel]]
    xt = np.ascontiguousarray(
        buf.reshape(E, NTMAX, P, 4, P).transpose(0, 4, 1, 3, 2)
    ).reshape(E, P, NTMAX * 512)

    # ---- weights: [E, O, I] -> W^T tile layout [E, 128 i_lo, 4 j * 512 o] ----
    wt16 = np.ascontiguousarray(weight.transpose(0, 2, 1)).astype(np.float16)
    wt16 = np.ascontiguousarray(
        wt16.reshape(E, 4, P, D).transpose(0, 2, 1, 3)
    ).reshape(E, P, 4 * D)

    in_maps = []
    for c in range(NC):
        ex = perm[:, c]
        in_maps.append(
            {
                "x": np.concatenate(
                    [xt[ex[k]][:, : nt_slot[k] * 512] for k in range(EL)], axis=1
                ),
                "wt": np.ascontiguousarray(
                    wt16[ex].transpose(1, 0, 2).reshape(P, EL * 4 * D)
                ),
                "bias": np.ascontiguousarray(
                    bias[ex].reshape(1, EL * D).astype(np.float16)
                ),
            }
        )

    trace = bool(int(os.environ.get("KERNEL_TRACE", "0")))
    kwargs = {}
    if trace:
        _ensure_ntff_hook()
        bass_utils.upload_artifacts = lambda tmpdir: "local://" + tmpdir
        tdir = os.environ.get("KERNEL_TRACE_DIR")
        if tdir:
            os.makedirs(tdir, exist_ok=True)
            kwargs["tmpdir"] = tdir
    res = run_bass_kernel_spmd(
        nc, in_maps, core_ids=list(range(NC)), trace=trace, **kwargs
    )
    last_result = res

    # ---- unpack: y blocks [128 t_lo, bt*512+o] per (slot, core) -> [E, CAPD, D]
    ypad = np.zeros((E, P, NTMAX * 512), np.float16)
    for c in range(NC):
        yc = res.results[c]["y"]
        for k in range(EL):
            ypad[perm[k, c]][:, : nt_slot[k] * 512] = yc[
                :, off[k] * 512 : (off[k] + nt_slot[k]) * 512
            ]
    yall = (
        ypad.reshape(E, P, NTMAX, D).transpose(0, 2, 1, 3).reshape(E, CAPD, D)
    )
    out = np.zeros((T, D), np.float32)
    out[order[sel]] = yall[ids_s[sel], pos_s[sel]].astype(np.float32)

    # tokens beyond device capacity but within global capacity: exact host math
    ovf = (~sel) & (pos_s < CAPG)
    for t_idx in order[ovf]:
        e = expert_ids[t_idx]
        out[t_idx] = weight[e] @ x[t_idx] + bias[e]
    return out
